# revision 37
# baseline (speedup 1.0000x reference)
import sys

for p in ("/opt/trn_rl_repo",):
    if p not in sys.path:
        sys.path.insert(0, p)

import numpy as np
import ml_dtypes
import jax

# Persistent XLA compilation cache: run_bass_via_pjrt builds a fresh jit
# closure per call, so the in-memory pjit cache always misses (~0.4s/call).
# The HLO embeds the full compressed BIR (ant_bir in backend_config), so the
# disk cache key uniquely identifies the kernel.
jax.config.update("jax_compilation_cache_dir", "/tmp/jax_comp_cache")
jax.config.update("jax_persistent_cache_min_compile_time_secs", 0.0)
jax.config.update("jax_persistent_cache_min_entry_size_bytes", 0)
jax.config.update("jax_raise_persistent_cache_errors", False)

import concourse.bass as bass
import concourse.mybir as mybir
import concourse.tile as tile
from concourse import bacc, bass_utils

# Problem dims (hardcoded per contract)
B, S, DM, H, Dh = 2, 4096, 2048, 16, 128
NCORES = 8
SL = (B * S) // NCORES      # 1024 positions per core
P = 128
KT = DM // P                # 16 contraction tiles
GP = 8                      # positions per attention group (8*16 heads = 128)
GB = 4                      # groups per attention batch
NB = SL // (GP * GB)        # 32 batches per core
NG = SL // GP               # 128 groups per core
INV_SQRT_D = 1.0 / float(np.sqrt(Dh))

BF = mybir.dt.bfloat16
NBF = ml_dtypes.bfloat16
AF = mybir.ActivationFunctionType
ALU = mybir.AluOpType


def _build_nc(reps=1):
    """Per-core fused kernel, all-bf16 datapath, no collectives.

    reps > 1 emits the whole body multiple times (same I/O) — used only for
    dispatch-overhead-cancelling timing; results are identical.

    Data-parallel over positions: core c owns 1024 rows of x (flattened
    [B*S, DM]). Full weights are shipped to every core.

    Inputs (per core):
      xt  [DM, SL]   bf16 — x shard transposed ([e, s])
      wt  [DM, 3*DM] bf16 — [Wq^T/sqrt(D) | Wk^T | Wv^T]  ([e, f])
      axb [1, DM]    bf16 — V bias (broadcast to all partitions on device)
      msk [P, P]     bf16 — 0/1 mask: msk[i, j] = 1 iff i//16 == j//16
      bqk [P, 32]    fp32 — Q/K bias: col m*16+t, row p = bias_m[t*128+p]
                            (Q bias pre-scaled by 1/sqrt(D))
    Output:
      out [NB, P, GB, Dh] bf16 — out[gb, gp*16+h, g, d] =
        attention output for position 32*gb + 8*g + gp, head h.
    """
    nc = bacc.Bacc(None, target_bir_lowering=False, num_devices=NCORES)
    xt = nc.dram_tensor("xt", [DM, SL], BF, kind="ExternalInput")
    wt = nc.dram_tensor("wt", [DM, 3 * DM], BF, kind="ExternalInput")
    axb = nc.dram_tensor("axb", [1, DM], BF, kind="ExternalInput")
    msk = nc.dram_tensor("msk", [P, P], BF, kind="ExternalInput")
    bqk = nc.dram_tensor("bqk", [P, 2 * H], mybir.dt.float32, kind="ExternalInput")
    out = nc.dram_tensor("out", [NB, P, GB, Dh], BF, kind="ExternalOutput")

    with tile.TileContext(nc) as tc:
        for rep in range(reps):
            _emit_body(nc, tc, rep, xt, wt, axb, msk, bqk, out)
    nc.finalize()
    return nc


def _emit_body(nc, tc, rep, xt, wt, axb, msk, bqk, out):
    r_ = f"r{rep}_"
    with tc.tile_pool(name=r_ + "dram", bufs=1, space="DRAM") as dram, \
         tc.tile_pool(name=r_ + "resident", bufs=1) as res:
            # V in attention-batch layout: vdram[gb, g, gp*16+t, d] =
            # V[position 32*gb+8*g+gp, head t, d]. This linearization equals
            # vsb's partition-major order, so the scatter is contiguous.
            vdram = dram.tile([NB, GB, P, Dh], BF)
            # ---- Resident SBUF tensors ----
            xts = res.tile([P, KT, SL], BF)           # x^T k-tiles
            # Q^T/K^T for the scores matmul: [d, group, (gp, h)]
            qts = res.tile([P, NG, GP, H], BF)
            kts = res.tile([P, NG, GP, H], BF)
            wvs = res.tile([P, KT, DM], BF)           # full V weights
            bvb = res.tile([P, DM], BF)               # V bias broadcast
            m01 = res.tile([P, P], BF)                # 0/1 mask
            ones_sb = res.tile([P, 1], BF)
            bqs = res.tile([P, 2 * H], mybir.dt.float32)

            # fine-grained x^T chunks (2 k-tiles each, ~0.73us) so the first
            # head's matmuls start ~2.5us in and never starve (PE consumes a
            # k-tile every ~0.43us; DMA delivers one every ~0.37us)
            def load_xts_chunk(kc, eng=None):
                (eng or nc.sync).dma_start(
                    xts[:, 2 * kc:2 * (kc + 1), :],
                    xt[256 * kc:256 * (kc + 1), :].rearrange(
                        "(k p) s -> p k s", p=P),
                )

            def load_aux():
                nc.sync.dma_start(bvb[:], axb[0:1, :].to_broadcast([P, DM]))
                nc.sync.dma_start(m01[:], msk[:, :])
                nc.any.memset(ones_sb[:], 1.0)

            load_xts_chunk(0, eng=nc.scalar)   # Act queue: overlaps strip0's
                                               # SP-queue fixed DMA latency

            # ---- Q^T / K^T projections: out[m=head dims, n=positions];
            # V-weight chunk loads interleaved between strips ----
            with tc.tile_pool(name=r_ + "wqk", bufs=2) as wpool, \
                 tc.tile_pool(name=r_ + "psum_qk", bufs=2, space="PSUM") as pp:
                for mat, dst in ((0, qts), (1, kts)):
                    for tp in range(H // 2):          # 2 heads per strip
                        col0 = mat * DM + tp * 256
                        strip = wpool.tile([P, KT, 256], BF, tag="wqk")
                        if mat == 0 and tp == 0:
                            # split the first strip so head 0's weights land
                            # before head 1's — trims PE startup idle
                            for hh in range(2):
                                nc.sync.dma_start(
                                    strip[:, :, 128 * hh:128 * (hh + 1)],
                                    wt[:, col0 + 128 * hh:col0 + 128 * (hh + 1)]
                                    .rearrange("(k p) d -> p k d", p=P),
                                )
                        else:
                            nc.sync.dma_start(
                                strip[:],
                                wt[:, col0:col0 + 256].rearrange(
                                    "(k p) d -> p k d", p=P),
                            )
                        if mat == 0 and tp == 0:
                            # bqs before the first activation consumes it
                            # (program order: after strip0, ~9us of slack)
                            nc.sync.dma_start(bqs[:], bqk[:, :])
                            for kc in range(1, 8):
                                load_xts_chunk(kc)
                        if mat == 0 and tp == 1:
                            load_aux()
                        if mat == 0 and 2 <= tp <= 5:  # wv chunk during QK
                            j = tp - 2
                            nc.sync.dma_start(
                                wvs[:, :, 512 * j:512 * (j + 1)],
                                wt[:, 2 * DM + 512 * j:2 * DM + 512 * (j + 1)]
                                .rearrange("(k p) f -> p k f", p=P),
                            )
                        for hh in range(2):
                            t = tp * 2 + hh
                            ps = pp.tile([P, SL], mybir.dt.float32, tag="ps")
                            for k in range(KT):
                                for j in range(2):
                                    nc.tensor.matmul(
                                        ps[:, 512 * j:512 * (j + 1)],
                                        strip[:, k, 128 * hh:128 * (hh + 1)],
                                        xts[:, k, 512 * j:512 * (j + 1)],
                                        start=(k == 0),
                                        stop=(k == KT - 1),
                                    )
                            c = mat * H + t
                            nc.scalar.activation(
                                dst[:, :, :, t],
                                ps[:].rearrange("p (g s) -> p g s", s=GP),
                                AF.Identity,
                                bias=bqs[:, c:c + 1],
                            )

            # ---- V projection interleaved with attention (software
            # pipeline: per V f-chunk, emit scores for one earlier batch
            # before and its out-matmuls after, so PE never waits on the
            # exp/mask chain) ----
            with tc.tile_pool(name=r_ + "vt", bufs=2) as vpool, \
                 tc.tile_pool(name=r_ + "psum_v", bufs=2, space="PSUM") as pv, \
                 tc.tile_pool(name=r_ + "attn", bufs=3) as ap_, \
                 tc.tile_pool(name=r_ + "psum_s", bufs=2, space="PSUM") as psp, \
                 tc.tile_pool(name=r_ + "psum_o", bufs=2, space="PSUM") as pop, \
                 tc.tile_pool(name=r_ + "psum_r", bufs=2, space="PSUM") as prp:

                state = {}

                def attn_scores(gb):
                    # V block gather + scores + exp + mask for batch gb
                    vblk = ap_.tile([P, GB, Dh], BF, tag="vblk")
                    nc.sync.dma_start(
                        vblk[:], vdram[gb].rearrange("g q d -> q g d"))
                    ps_s = psp.tile([P, GB, P], mybir.dt.float32, tag="ps_s")
                    for gi in range(GB):
                        nc.tensor.matmul(
                            ps_s[:, gi, :],
                            kts[:, gb * GB + gi],
                            qts[:, gb * GB + gi],
                            start=True, stop=True,
                        )
                    # exp then 0/1-mask multiply (exp(s+m) == exp(s)*[m==0])
                    e_t = ap_.tile([P, GB, P], BF, tag="e_t")
                    nc.scalar.activation(e_t[:], ps_s[:], AF.Exp)
                    nc.vector.tensor_tensor(
                        e_t[:], e_t[:],
                        m01[:, None, :].to_broadcast([P, GB, P]),
                        ALU.mult,
                    )
                    state[gb] = (vblk, e_t)

                def attn_out(gb):
                    vblk, e_t = state.pop(gb)
                    ps_o = pop.tile([P, GB, Dh], mybir.dt.float32, tag="ps_o")
                    ps_r = prp.tile([P, GB], mybir.dt.float32, tag="ps_r")
                    for gi in range(GB):
                        nc.tensor.matmul(
                            ps_o[:, gi, :], e_t[:, gi, :], vblk[:, gi, :],
                            start=True, stop=True,
                        )
                        nc.tensor.matmul(
                            ps_r[:, gi:gi + 1], e_t[:, gi, :], ones_sb[:],
                            start=True, stop=True,
                        )
                    rc = ap_.tile([P, GB], mybir.dt.float32, tag="rc")
                    nc.vector.reciprocal(rc[:], ps_r[:])
                    ob = ap_.tile([P, GB, Dh], BF, tag="ob")
                    nc.vector.tensor_tensor(
                        ob[:], ps_o[:],
                        rc[:, :, None].to_broadcast([P, GB, Dh]),
                        ALU.mult,
                    )
                    nc.sync.dma_start(out[gb], ob[:])

                for mt in range(SL // P):
                    vtile = vpool.tile([P, DM], BF, tag="vt")
                    for j in range(4):
                        if mt >= 1:
                            attn_scores(4 * (mt - 1) + j)
                        ps = pv.tile([P, 512], mybir.dt.float32, tag="psv")
                        for k in range(KT):
                            nc.tensor.matmul(
                                ps[:],
                                xts[:, k, P * mt:P * (mt + 1)],
                                wvs[:, k, 512 * j:512 * (j + 1)],
                                start=(k == 0),
                                stop=(k == KT - 1),
                            )
                        # copy + V bias + cast in one DVE op
                        nc.vector.tensor_tensor(
                            vtile[:, 512 * j:512 * (j + 1)],
                            ps[:],
                            bvb[:, 512 * j:512 * (j + 1)],
                            ALU.add,
                        )
                        if mt >= 1:
                            attn_out(4 * (mt - 1) + j)
                    # scatter V tile: contiguous (dst linear order == src
                    # partition-major order: i, g, gp, t, d)
                    nc.sync.dma_start(vdram[4 * mt:4 * (mt + 1)], vtile[:])

                # drain: last position tile's 4 batches
                for j in range(4):
                    attn_scores(28 + j)
                for j in range(4):
                    attn_out(28 + j)


_NC_CACHE = {}


def _get_nc(reps=1):
    if reps not in _NC_CACHE:
        _NC_CACHE[reps] = _build_nc(reps)
    return _NC_CACHE[reps]


def build_in_maps(x, Wq, bq, Wk, bk, Wv, bv):
    """Host-side prep: transpose/shard/cast to bf16."""
    x = np.asarray(x, np.float32).reshape(NCORES, SL, DM)
    xts = [np.ascontiguousarray(x[c].T).astype(NBF) for c in range(NCORES)]

    WqT = (np.asarray(Wq, np.float32) * INV_SQRT_D).T     # [e, f]
    WkT = np.asarray(Wk, np.float32).T
    WvT = np.asarray(Wv, np.float32).T
    wt = np.ascontiguousarray(
        np.concatenate([WqT, WkT, WvT], axis=1).astype(NBF))  # [DM, 3*DM]

    axb = np.asarray(bv, np.float32).astype(NBF).reshape(1, DM)

    blk = np.arange(P) // H
    msk = (blk[:, None] == blk[None, :]).astype(NBF)          # [P, P]

    bqk = np.zeros((P, 2 * H), np.float32)
    bq_s = np.asarray(bq, np.float32) * INV_SQRT_D
    bk_f = np.asarray(bk, np.float32)
    p = np.arange(P)
    for t in range(H):
        bqk[:, t] = bq_s[t * Dh + p]
        bqk[:, H + t] = bk_f[t * Dh + p]

    in_maps = []
    for c in range(NCORES):
        in_maps.append(
            {"xt": xts[c], "wt": wt, "axb": axb, "msk": msk, "bqk": bqk})
    return in_maps


def kernel(x, Wq, bq, Wk, bk, Wv, bv):
    nc = _get_nc()
    in_maps = build_in_maps(x, Wq, bq, Wk, bk, Wv, bv)
    try:
        res = bass_utils.run_bass_kernel_spmd(
            nc, in_maps, core_ids=list(range(NCORES)))
    except Exception:
        # transient NRT device errors (e.g. NRT_EXEC_UNIT_UNRECOVERABLE)
        # usually clear on retry
        import time as _time
        _time.sleep(5)
        res = bass_utils.run_bass_kernel_spmd(
            nc, in_maps, core_ids=list(range(NCORES)))

    # out[gb, gp*16+h, g, d] = F-value for position 32*gb+8*g+gp, head h
    F = np.empty((B, H, S, Dh), np.float32)
    for c in range(NCORES):
        b, sc = c // (NCORES // B), c % (NCORES // B)
        o = np.asarray(res.results[c]["out"]).astype(np.float32)
        o = o.reshape(NB, GP, H, GB, Dh)                 # [gb, gp, h, g, d]
        F[b, :, SL * sc:SL * (sc + 1), :] = (
            o.transpose(2, 0, 3, 1, 4).reshape(H, SL, Dh)
        )
    return F.reshape(B, S, H * Dh)
